# revision 14
# baseline (speedup 1.0000x reference)
"""Trainium2 Bass kernel for nn_ComplexMixture.

Reference:
  output_real[b,n,m] = sum_s w[b,s] * (r[b,s,n]*r[b,s,m] + i[b,s,n]*i[b,s,m])
  output_imag[b,n,m] = sum_s w[b,s] * (i[b,s,n]*r[b,s,m] - r[b,s,n]*i[b,s,m])

Shapes: B=32, S=128, N=256, fp32. w is uniform [0,1) so sqrt(w) is real.

Data-parallel over B across 8 cores, 4 batches/core. Host packs r|i into
one input tensor [BPC, 2, S, N] and sqrt-weights into swn=[sqrt(w).T | -sqrt(w).T];
device writes one output tensor [BPC, 2, 2, 128, N] which the host reassembles.

Per batch on one core (S=128 = partition/contraction dim):
  X  = [r | i]             [128, 512]   one DMA
  Y  = sqrt(w)[:,None]*X   [128, 512]   DVE per-partition scalar; rounds to mm dtype
  Yn = -sqrt(w)[:,None]*r  [128, 256]   ACT scaled copy
  ps_r[:, c*256:+256] = Yr_c.T @ Yr + Yi_c.T @ Yi   (PSUM accumulation, c=0,1)
  ps_i[:, c*256:+256] = Yi_c.T @ Yr + Yn_c.T @ Yi
  O[:, 0:512]    = copy ps_r (DVE)
  O[:, 512:1024] = copy ps_i (ACT)
  one DMA: O -> out[b]  ([t c] merged AP)
"""

import os

import numpy as np

import concourse.bass as bass
import concourse.mybir as mybir
import concourse.tile as tile
from concourse import bacc
from concourse.bass_utils import run_bass_kernel_spmd

B, S, N = 32, 128, 256
NCORES = 8
BPC = B // NCORES  # batches per core

F32 = mybir.dt.float32
# Matmul operand dtype: float32r streams at 1 cycle/row (vs 4 for float32).
MM_DT = mybir.dt.float32r if os.environ.get("CM_MM_F32R", "1") == "1" else F32

LAST_RESULTS = None  # stashed BassKernelResults for test harness introspection


def build_nc() -> bass.Bass:
    nc = bacc.Bacc()
    xin = nc.dram_tensor("xin", [BPC, 2, S, N], F32, kind="ExternalInput")
    w = nc.dram_tensor("swn", [S, 2 * BPC], F32, kind="ExternalInput")
    out = nc.dram_tensor("out_all", [BPC, 2, 2, 128, N], F32, kind="ExternalOutput")

    with tile.TileContext(nc) as tc:
        with (
            tc.tile_pool(name="io", bufs=BPC) as io_pool,
            tc.tile_pool(name="wp", bufs=1) as w_pool,
            tc.tile_pool(name="op", bufs=BPC) as out_pool,
            tc.tile_pool(name="ps", bufs=3, space="PSUM") as ps_pool,
        ):
            swn = w_pool.tile([S, 2 * BPC], F32, tag="swn", name="swn")
            nc.sync.dma_start(out=swn, in_=w[:, :])
            sw = swn[:, 0:BPC]
            nsw = swn[:, BPC : 2 * BPC]
            # Prime the ACT function table early so the ~1.3us table load
            # overlaps the input DMAs instead of stalling the first copy.
            prime = w_pool.tile([S, 1], F32, tag="prime", name="prime")
            nc.scalar.copy(out=prime, in_=swn[:, 0:1])

            # All input loads up front; no deps, so they stream back-to-back.
            Xs = []
            for b in range(BPC):
                X = io_pool.tile([S, 2 * N], F32, tag="X", name=f"X{b}")
                nc.sync.dma_start(
                    out=X.rearrange("s (t n) -> s t n", t=2),
                    in_=xin[b].rearrange("t s n -> s t n"),
                )
                Xs.append(X)

            for b in range(BPC):
                X = Xs[b]
                Y = io_pool.tile([S, 2 * N], MM_DT, tag="Y", name=f"Y{b}")
                nc.vector.tensor_scalar_mul(Y, X, sw[:, b : b + 1])
                Yn = io_pool.tile([S, N], MM_DT, tag="Yn", name=f"Yn{b}")
                nc.scalar.activation(
                    out=Yn, in_=X[:, 0:N],
                    func=mybir.ActivationFunctionType.Copy,
                    scale=nsw[:, b : b + 1],
                )

                Yr = Y[:, 0:N]
                Yi = Y[:, N : 2 * N]
                ps_r = ps_pool.tile([128, 2 * N], F32, tag="psR", name=f"psR{b}")
                ps_i = ps_pool.tile([128, 2 * N], F32, tag="psI", name=f"psI{b}")
                for c in range(2):
                    csl = slice(c * 128, c * 128 + 128)
                    osl = slice(c * N, (c + 1) * N)
                    nc.tensor.matmul(ps_r[:, osl], lhsT=Yr[:, csl], rhs=Yr, start=True, stop=False)
                    nc.tensor.matmul(ps_r[:, osl], lhsT=Yi[:, csl], rhs=Yi, start=False, stop=True)
                    nc.tensor.matmul(ps_i[:, osl], lhsT=Yi[:, csl], rhs=Yr, start=True, stop=False)
                    nc.tensor.matmul(ps_i[:, osl], lhsT=Yn[:, csl], rhs=Yi, start=False, stop=True)

                O = out_pool.tile([128, 4 * N], F32, tag="O", name=f"O{b}")
                nc.vector.tensor_copy(O[:, 0 : 2 * N], ps_r)
                nc.scalar.copy(out=O[:, 2 * N : 4 * N], in_=ps_i)
                # out[b, t, c, p, m] <- O[p, (t c m)]
                dst = out[b].rearrange("t c p m -> p (t c) m")
                src = O.rearrange("p (tc m) -> p tc m", m=N)
                nc.sync.dma_start(out=dst, in_=src)
    nc.compile()
    return nc


def kernel(**inputs: np.ndarray):
    global LAST_RESULTS
    r = np.asarray(inputs["input_real"], dtype=np.float32)
    i = np.asarray(inputs["input_imag"], dtype=np.float32)
    w = np.ascontiguousarray(np.asarray(inputs["weight"], dtype=np.float32))
    assert r.shape == (B, S, N) and i.shape == (B, S, N) and w.shape == (B, S)

    xin = np.ascontiguousarray(np.stack([r, i], axis=1))  # [B, 2, S, N]
    sws = np.sqrt(w)  # [B, S]

    in_maps = []
    for c in range(NCORES):
        sl = slice(c * BPC, (c + 1) * BPC)
        in_maps.append(
            {
                "xin": np.ascontiguousarray(xin[sl]),
                "swn": np.ascontiguousarray(
                    np.concatenate([sws[sl].T, -sws[sl].T], axis=1)
                ),
            }
        )

    nc = build_nc()
    res = run_bass_kernel_spmd(nc, in_maps, core_ids=list(range(NCORES)))
    LAST_RESULTS = res

    out_all = np.concatenate(
        [res.results[c]["out_all"] for c in range(NCORES)], axis=0
    )  # [B, 2, 2, 128, N]
    out_all = out_all.reshape(B, 2, N, N)
    return (np.ascontiguousarray(out_all[:, 0]), np.ascontiguousarray(out_all[:, 1]))
